# revision 29
# baseline (speedup 1.0000x reference)
"""Trainium2 Bass kernel for nn_AtlasMultiDiffAttn (8-core data-parallel).

v3: interleaved pair-minor layouts (no dup-shift images), partition-major
single-DMA inputs (2 big contiguous DMAs per tile vs 12 small ones),
batched softmax phases (4x fewer act-table loads), engine rebalancing
across ACT/DVE/Pool (GPSIMD only touches SBUF -- it cannot access PSUM).

Self-contained: hardcodes shapes (x [8192,56,128] f32 -> out [8192,56] f32).
Per core: 1024 samples, 8 tiles of BT=128 samples (64 even/odd sample pairs).

Host-side packing (both partition-major, contiguous per-partition tile
lines, pair index q innermost so DoubleRow tap pairs are stride-64 column
neighbours -- no duplicated shift images):
  - xq8 fp8 [128p, NT, 136c, 64q]: p 0-55 even-sample atlas ch a, 64-119
    odd; c = e + 3 (3 zero pad each side); value x[b(q,par), a, c-3]
  - xtp bf16 [128e, npairs, 128]: cols 0-55 even-sample a, 64-119 odd

Math folds:
  - k-LN mean folded into w_k (per-head row-mean removed) => mean(k)=0
  - convs in fp8e4m3 DoubleRow, tap t = 2*jp + s where s is the DR slot
    reading column c vs c+1 (stride 64 >= 16B, satisfying DR step rule)
  - rsqrt via magic-init + 1 Newton iteration on DVE
"""
from contextlib import ExitStack

import numpy as np

import concourse.bass as bass
import concourse.tile as tile
from concourse import bacc, mybir
from concourse.bass_utils import run_bass_kernel_spmd

F32 = mybir.dt.float32
BF16 = mybir.dt.bfloat16
FP8 = mybir.dt.float8e4
I32 = mybir.dt.int32
AF = mybir.ActivationFunctionType
OP = mybir.AluOpType
AX = mybir.AxisListType
PM = mybir.MatmulPerfMode

B, A, E = 8192, 56, 128
H, HD = 4, 16
LAMBDA_INIT = 0.7
EPS = 1e-5
SCALING = HD ** -0.5
EPS_Q = float(A) ** 2 * EPS
WSCALE = 16.0

NCORES = 8
NB = B // NCORES          # 1024 samples per core
BT = 128                  # samples per tile
NT = NB // BT             # 8 tiles
NPAIR = BT // 2           # 64
CC = 136                  # padded conv1 image cols (e + 3 each side, /16)
MAGIC = 0x5F3759DF

WSPEC = {}


def _newton_rsqrt(nc, pool, v_ap, shape, tag):
    """v_ap <- rsqrt(v_ap): magic init + 1 Newton iteration (~0.2% rel)."""
    y = pool.tile(list(shape), F32, tag=f"nwy_{tag}")
    t = pool.tile(list(shape), F32, tag=f"nwt_{tag}")
    npart = v_ap.shape[0]
    ya, ta = y[0:npart], t[0:npart]
    nc.vector.tensor_scalar(out=ya.bitcast(I32), in0=v_ap.bitcast(I32),
                            scalar1=1, scalar2=None,
                            op0=OP.logical_shift_right)
    nc.vector.tensor_scalar(out=ya.bitcast(I32), in0=ya.bitcast(I32),
                            scalar1=-1, scalar2=MAGIC,
                            op0=OP.mult, op1=OP.add)
    nc.vector.tensor_tensor(out=ta, in0=ya, in1=ya, op=OP.mult)
    nc.vector.tensor_tensor(out=ta, in0=ta, in1=v_ap, op=OP.mult)
    nc.vector.tensor_scalar(out=ta, in0=ta, scalar1=-0.5, scalar2=1.5,
                            op0=OP.mult, op1=OP.add)
    nc.vector.tensor_tensor(out=v_ap, in0=ya, in1=ta, op=OP.mult)


def apx(base, offset_add, dims):
    """Raw AP with explicit free dims, keeping base's partition dim."""
    return bass.AP(tensor=base.tensor, offset=base.offset + offset_add,
                   ap=[list(base.ap[0])] + [list(d) for d in dims])


def build_tile_kernel(ctx, tc, x8_ext, xt_ext, out_ext, wext, repeat=1):
    nc = tc.nc

    consts = ctx.enter_context(tc.tile_pool(name="consts", bufs=1))
    sbA = ctx.enter_context(tc.tile_pool(name="sbA", bufs=1))
    sbX = ctx.enter_context(tc.tile_pool(name="sbX", bufs=2))
    sbK = ctx.enter_context(tc.tile_pool(name="sbK", bufs=1))
    sbQ = ctx.enter_context(tc.tile_pool(name="sbQ", bufs=2))
    sbS = ctx.enter_context(tc.tile_pool(name="sbS", bufs=4))
    sb2 = ctx.enter_context(tc.tile_pool(name="sb2", bufs=2))
    psA = ctx.enter_context(tc.tile_pool(name="psA", bufs=4, space="PSUM"))
    psK = ctx.enter_context(tc.tile_pool(name="psK", bufs=2, space="PSUM"))
    psC = ctx.enter_context(tc.tile_pool(name="psC", bufs=2, space="PSUM"))

    def cload(name):
        shape, pdt = WSPEC[name]
        t = consts.tile(list(shape), pdt, tag=f"c_{name}")
        nc.sync.dma_start(out=t[:], in_=wext[name][:])
        return t

    w1 = cload("w1")            # [128, 4, 2, 128] fp8
    w2 = cload("w2")            # [128, 4, 2, 128] fp8
    wk = cload("wkT")           # [128, 128] bf16 (head-mean folded)
    g64 = cload("g64")          # [128, 64] bf16
    qg = cload("qG")            # [128, 16] bf16
    expd = cload("expand")      # [8, 128] f32
    id128 = cload("ident128b")  # [128, 128] bf16
    idb64 = cload("ident64b")   # [64, 64] bf16
    c1 = cload("c1")            # [128, 1] f32
    c2 = cload("c2")
    lamrow = cload("lamrow")    # [128, 8] f32

    state = {}
    state2 = {}

    def phaseA(it):
        xq = sbX.tile([128, CC, 64], FP8, tag="xq")
        nc.sync.dma_start(out=xq[:], in_=x8_ext[:, it])
        xt = sbX.tile([128, NPAIR, 128], BF16, tag="xt")
        nc.sync.dma_start(out=xt[:],
                          in_=xt_ext[:, it * NPAIR:(it + 1) * NPAIR, :])

        # ---- conv1 fp8 DoubleRow + native silu -> h8e [p,m / e,q] ----
        # jp-outer over blocks of 4 groups: consecutive matmuls share the
        # same stationary weights (weight-stationary LDW reuse)
        h8e = sbA.tile([128, 128, 64], BF16, tag="h8e")
        b0 = xq[:, 0:1, 0:1]
        for blk in range(4):
            pss = []
            for _gi in range(4):
                ps = psA.tile([128, 512], F32, tag="ps")
                pss.append(ps)
            for jp in range(4):
                for gi in range(4):
                    eb = blk * 4 + gi
                    rhs = bass.AP(
                        tensor=b0.tensor,
                        offset=b0.offset + (8 * eb + 2 * jp) * 64,
                        ap=[[list(b0.ap[0])[0], 120], [64, 2],
                            [64, 8], [1, 64]])
                    nc.tensor.matmul(pss[gi][:], w1[0:120, jp, :, :], rhs,
                                     start=(jp == 0), stop=(jp == 3),
                                     perf_mode=PM.DoubleRow)
            for gi in range(4):
                eb = blk * 4 + gi
                nc.scalar.activation(
                    h8e[:, 8 * eb:8 * eb + 8, :]
                        .rearrange("p e q -> p (e q)"),
                    pss[gi][:], AF.Silu, scale=1.0 / WSCALE)

        # ---- h transposes -> hT8 [e, m, q] (no dup; q innermost) -----
        hT8 = sbA.tile([128, 128, 64], FP8, tag="hT8")
        for g in range(8):
            psx = psC.tile([128, 8, 128], BF16, tag="psb")
            for j in range(8):
                nc.tensor.transpose(psx[:, j, :], h8e[:, :, 8 * g + j],
                                    id128[:])
            outw = apx(hT8[:, 0:1, 0:1], 8 * g, [[64, 128], [1, 8]])
            inw = apx(psx[:, 0:1, 0:1], 0, [[1, 128], [128, 8]])
            if g % 2 == 0:
                nc.vector.tensor_copy(out=outw, in_=inw)
            else:
                nc.scalar.copy(out=outw, in_=inw)

        # ---- conv2 fp8 DoubleRow + silu -> h2s [p, par, a, q] --------
        # weight-stationary blocks, as in conv1
        h2s = sbA.tile([128, 2, 56, 64], BF16, tag="h2s")
        b1 = hT8[:, 0:1, 0:1]
        groups = [(ab, par) for ab in range(7) for par in range(2)]
        for blk in range(0, 14, 4):
            gblk = groups[blk:blk + 4]
            pss = []
            for _gi in gblk:
                ps = psA.tile([128, 512], F32, tag="ps")
                pss.append(ps)
            for jp in range(4):
                for gi, (ab, par) in enumerate(gblk):
                    rhs = bass.AP(
                        tensor=b1.tensor,
                        offset=b1.offset
                        + (8 * ab + 2 * jp + 64 * par) * 64,
                        ap=[[list(b1.ap[0])[0], 128], [64, 2],
                            [64, 8], [1, 64]])
                    nc.tensor.matmul(pss[gi][:], w2[:, jp, :, :], rhs,
                                     start=(jp == 0), stop=(jp == 3),
                                     perf_mode=PM.DoubleRow)
            for gi, (ab, par) in enumerate(gblk):
                nc.scalar.activation(
                    h2s[:, par, 8 * ab:8 * ab + 8, :]
                        .rearrange("p a q -> p (a q)"),
                    pss[gi][:], AF.Silu, scale=1.0 / WSCALE)

        # ---- q_acc[p, n=2q+par] = sum_a h2s (split for engine balance)
        q_acc = sbQ.tile([128, 128], BF16, tag="qacc")
        rha = sbQ.tile([128, 2, 2, 64], BF16, tag="rha")
        with nc.allow_low_precision(reason="56-term sum; LN follows"):
            for par in range(2):
                for half in range(2):
                    a0 = 28 * half
                    eng = nc.vector
                    eng.reduce_sum(
                        rha[:, par, half, :],
                        h2s[:, par, a0:a0 + 28, :].transpose([0, 2, 1]),
                        axis=AX.X)
            for par in range(2):
                nc.vector.tensor_tensor(
                    out=apx(q_acc[:, 0:1], par, [[2, 64]]),
                    in0=rha[:, par, 0, :], in1=rha[:, par, 1, :],
                    op=OP.add)

        state[it] = (xt, q_acc)

    def phaseB1(it):
        xt, q_acc = state.pop(it)
        # ---- q-side stats/LN (psum segments: muq 0:128 then reused for
        # expd-vq; sumq2 128:256; expd-mu 256:384) ---------------------
        qpsA = psK.tile([128, 448], F32, tag="pk")
        nc.tensor.matmul(qpsA[0:8, 0:128], qg[:, 0:8], q_acc[:],
                         start=True, stop=True)                 # muq
        q2 = sbQ.tile([128, 128], BF16, tag="q2")
        nc.vector.tensor_tensor(out=q2[:], in0=q_acc[:], in1=q_acc[:],
                                op=OP.mult)
        nc.tensor.matmul(qpsA[0:8, 128:256], qg[:, 8:16], q2[:],
                         start=True, stop=True)                 # sum q^2
        muq = sbQ.tile([128, 128], F32, tag="muq")
        nc.vector.tensor_copy(out=muq[0:8, :], in_=qpsA[0:8, 0:128])
        vq = sbQ.tile([128, 128], F32, tag="vq")
        nc.vector.tensor_tensor(out=vq[0:8, :], in0=muq[0:8, :],
                                in1=muq[0:8, :], op=OP.mult)
        nc.vector.scalar_tensor_tensor(
            out=vq[0:8, :], in0=qpsA[0:8, 128:256], scalar=1.0 / HD,
            in1=vq[0:8, :], op0=OP.mult, op1=OP.subtract)
        nc.vector.tensor_scalar_add(vq[0:8, :], vq[0:8, :], EPS_Q)
        _newton_rsqrt(nc, sbQ, vq[0:8, :], [128, 128], "rq")
        nc.tensor.matmul(qpsA[:, 256:384], expd[:], muq[0:8, :],
                         start=True, stop=True)
        nc.tensor.matmul(qpsA[:, 0:128], expd[:], vq[0:8, :],
                         start=True, stop=True)
        qhat = sbQ.tile([128, 128], F32, tag="qhat")
        nc.vector.tensor_tensor(out=qhat[:], in0=q_acc[:],
                                in1=qpsA[:, 256:384], op=OP.subtract)
        nc.vector.tensor_tensor(out=qhat[:], in0=qhat[:],
                                in1=qpsA[:, 0:128], op=OP.mult)
        qb = sbQ.tile([128, 128], F32, tag="qb")
        nc.vector.tensor_scalar(out=qb[:], in0=qhat[:], scalar1=c1[:],
                                scalar2=c2[:], op0=OP.mult, op1=OP.add)
        qbb = sbQ.tile([128, 128], BF16, tag="qbb")
        nc.gpsimd.tensor_copy(out=qbb[:], in_=qb[:])

        # ---- k projection chunks -> k_T SBUF; k2/qbk mostly on Pool --
        k_T = sbK.tile([128, NPAIR, 112], BF16, tag="kT")
        k2 = sbK.tile([128, NPAIR, 112], BF16, tag="k2")
        qbk = sbK.tile([128, NPAIR, 2, A], BF16, tag="qbk")
        for ch in range(16):
            q0 = ch * 4
            pk = psK.tile([128, 448], F32, tag="pk")
            krhs = apx(xt[:, 0:1, 0:1], q0 * 128,
                       [[128, 4], [64, 2], [1, A]])
            nc.tensor.matmul(pk[:, 0:448], wk[:], krhs, start=True,
                             stop=True)
            kslice = k_T[:, q0:q0 + 4, :].rearrange("p q r -> p (q r)")
            if ch % 2 == 0:
                nc.vector.tensor_copy(out=kslice, in_=pk[:, 0:448])
            else:
                nc.scalar.copy(out=kslice, in_=pk[:, 0:448])
            k2o = k2[:, q0:q0 + 4, :].rearrange("p q r -> p (q r)")
            ksb = k_T[:, q0:q0 + 4, :].rearrange("p q r -> p (q r)")
            k2e = nc.vector if ch % 4 == 0 else nc.gpsimd
            k2e.tensor_tensor(out=k2o, in0=ksb, in1=ksb, op=OP.mult)
            qbe = nc.vector if ch % 4 == 2 else nc.gpsimd
            qbe.tensor_tensor(
                out=qbk[:, q0:q0 + 4, :, :],
                in0=k_T[:, q0:q0 + 4, :].rearrange("p q (c a) -> p q c a",
                                                   c=2),
                in1=qbb[:, 2 * q0:2 * q0 + 8]
                    .rearrange("p (q c) -> p q c", c=2).unsqueeze(3)
                    .to_broadcast((128, 4, 2, A)),
                op=OP.mult)

        # ---- k stats: 2 col-tiled matmuls per 4-pair group -----------
        stats_sb = sb2.tile([64, NPAIR, 112], BF16, tag="statsb")
        st_engs = ([nc.vector] * 3 + [nc.scalar] * 1) * 4
        for cki in range(16):
            p0 = cki * 4
            pst = psK.tile([128, 448], F32, tag="pk")
            nc.tensor.matmul(pst[0:32, 0:448], g64[:, 0:32],
                             k2[:, p0:p0 + 4, :], start=True, stop=True)
            nc.tensor.matmul(
                pst[32:64, 0:448], g64[:, 32:64],
                qbk[:, p0:p0 + 4, :, :].rearrange("p q c a -> p q (c a)"),
                start=True, stop=True, tile_position=(0, 32))
            so = stats_sb[0:64, p0:p0 + 4, :].rearrange("p q r -> p (q r)")
            eng = st_engs[cki]
            if eng is nc.scalar:
                nc.scalar.copy(out=so, in_=pst[0:64, 0:448])
            else:
                eng.tensor_copy(out=so, in_=pst[0:64, 0:448])

        # ---- per-l transposes -> statsB [128=n, 56, 2, 8] bf16 -------
        statsB = sbS.tile([128, A, 2, 8], BF16, tag="statsB")
        svb = stats_sb[:].rearrange("p q (c l) -> p (q c) l", c=2)
        for li in range(4):
            l0 = li * 14
            pstb = psC.tile([128, 14, 64], BF16, tag="psb")
            for j in range(14):
                nc.tensor.transpose(pstb[:, j, :], svb[0:64, :, l0 + j],
                                    idb64[:])
            inw = apx(pstb[:, 0:1, 0:1], 0, [[64, 14], [32, 2], [1, 8]])
            if li % 2 == 0:
                nc.vector.tensor_copy(out=statsB[:, l0:l0 + 14, :, :],
                                      in_=inw)
            else:
                nc.scalar.copy(out=statsB[:, l0:l0 + 14, :, :], in_=inw)

        state2[it] = statsB

    def phaseB2(it):
        statsB = state2.pop(it)
        # ---- score assembly on [128, 56, 8] --------------------------
        sk2 = statsB[:, :, 0, :]
        QK = statsB[:, :, 1, :]
        vk = sb2.tile([128, A, 8], F32, tag="vk")
        nc.vector.tensor_scalar(out=vk[:], in0=sk2, scalar1=1.0 / HD,
                                scalar2=EPS, op0=OP.mult, op1=OP.add)
        _newton_rsqrt(nc, sb2, vk[:], [128, A, 8], "rk")
        s_sc = sb2.tile([128, A, 8], F32, tag="ssc")
        nc.vector.tensor_tensor(out=s_sc[:], in0=QK, in1=vk[:], op=OP.mult)

        # ---- softmax1, diff, softmax2, mean over heads ---------------
        # scores are bounded (|s| <= |qb_h|*4 = O(4) by Cauchy-Schwarz):
        # exp cannot overflow, so skip the max-subtraction pass.
        nc.scalar.activation(s_sc[:], s_sc[:], AF.Exp)
        z1 = sb2.tile([128, 8], F32, tag="z1")
        nc.vector.reduce_sum(z1[:], s_sc[:].transpose([0, 2, 1]), axis=AX.X)
        rz1 = sb2.tile([128, 8], F32, tag="rz1")
        nc.vector.reciprocal(rz1[:], z1[:])
        nc.vector.tensor_tensor(out=rz1[:], in0=rz1[:], in1=lamrow[:],
                                op=OP.mult)
        nc.vector.tensor_tensor(
            out=s_sc[:], in0=s_sc[:],
            in1=rz1[:].unsqueeze(1).to_broadcast((128, A, 8)), op=OP.mult)
        dd = sb2.tile([128, A, 4], F32, tag="dd")
        nc.vector.tensor_tensor(out=dd[:], in0=s_sc[:, :, 0:8:2],
                                in1=s_sc[:, :, 1:8:2], op=OP.subtract)
        # diff entries lie in [-|lam|, 1]: exp never overflows, and the
        # softmax is shift-invariant, so skip the max pass entirely.
        nc.scalar.activation(dd[:], dd[:], AF.Exp)
        z2 = sb2.tile([128, 4], F32, tag="z2")
        nc.vector.reduce_sum(z2[:], dd[:].transpose([0, 2, 1]), axis=AX.X)
        rz2 = sb2.tile([128, 4], F32, tag="rz2")
        nc.vector.reciprocal(rz2[:], z2[:])
        nc.vector.tensor_scalar_mul(rz2[:], rz2[:], 1.0 / H)
        nc.vector.tensor_tensor(
            out=dd[:], in0=dd[:],
            in1=rz2[:].unsqueeze(1).to_broadcast((128, A, 4)), op=OP.mult)
        ot = sb2.tile([128, A], F32, tag="ot")
        nc.vector.reduce_sum(ot[:], dd[:], axis=AX.X)

        nc.sync.dma_start(out=out_ext[it * BT:(it + 1) * BT, :], in_=ot[:])

    for _rep in range(repeat):
        phaseA(0)
        phaseA(1)
        pend = []
        for it in range(NT):
            phaseB1(it)
            pend.append(it)
            if len(pend) == 4:
                for t in pend:
                    phaseB2(t)
                pend = []
            if it + 2 < NT:
                phaseA(it + 2)


def build_nc(repeat=1):
    nc = bacc.Bacc(target_bir_lowering=False, trn_type="TRN2")
    x8_ext = nc.declare_dram_parameter("x8", [128, NT, CC, 64], FP8,
                                       isOutput=False)
    xt_ext = nc.declare_dram_parameter("xt", [128, NB // 2, 128], BF16,
                                       isOutput=False)
    out_ext = nc.declare_dram_parameter("out", [NB, A], F32, isOutput=True)
    wext = {}
    for name, (shape, dt) in WSPEC.items():
        wext[name] = nc.declare_dram_parameter(name, list(shape), dt,
                                               isOutput=False)
    with tile.TileContext(nc) as tc:
        with ExitStack() as ctx:
            build_tile_kernel(ctx, tc, x8_ext, xt_ext, out_ext, wext,
                              repeat=repeat)
    nc.compile()
    return nc


def prepare_weights(w_emb, b_emb, w_atlas, b_atlas, w_k, qn_w, qn_b, kn_w,
                    kn_b, lambda_q1, lambda_k1, lambda_q2, lambda_k2):
    import ml_dtypes
    bf = ml_dtypes.bfloat16
    f8 = ml_dtypes.float8_e4m3fn
    f32 = np.float32

    assert np.allclose(b_atlas, 0.0), "kernel assumes b_atlas == 0"
    assert np.allclose(b_emb, 0.0), "kernel assumes b_emb == 0"
    assert np.allclose(kn_b, 0.0), "kernel assumes kn_b == 0"

    # conv1 lhsT [120, 4jp, 2s, 128]: tap 2jp+s; even block cols 3:59,
    # odd block cols 67:123 (the m output partition layout)
    w1 = np.zeros((128, 4, 2, 128), f32)
    w2 = np.zeros((128, 4, 2, 128), f32)
    for t in range(7):
        jp, s = t // 2, t % 2
        blk = np.transpose(w_emb[:, :, t]).astype(f32) * WSCALE
        w1[0:56, jp, s, 3:59] = blk
        w1[64:120, jp, s, 67:123] = blk
        w2[:, jp, s, :] = np.transpose(w_atlas[:, :, t]).astype(f32) * WSCALE

    # k projection with per-head row mean folded out (=> mean_h(k) == 0)
    wkf = np.asarray(w_k, f32)
    wkp = wkf - wkf.reshape(2 * H, HD, E).mean(axis=1, keepdims=True).repeat(
        HD, axis=1).reshape(E, E)
    wkT = np.ascontiguousarray(np.transpose(wkp)).astype(bf)

    G = np.zeros((128, 8), f32)
    for o in range(128):
        G[o, o // HD] = 1.0
    Z24 = np.zeros((128, 24), f32)
    g64 = np.ascontiguousarray(np.concatenate(
        [G, Z24, G, Z24], axis=1)).astype(bf)

    qG = np.ascontiguousarray(
        np.concatenate([G / HD, G], axis=1)).astype(bf)

    expand = np.ascontiguousarray(G.T).astype(f32)            # [8, 128]
    ident128b = np.eye(128, dtype=bf)
    ident64b = np.eye(64, dtype=bf)

    d_idx = np.arange(E) % HD
    c1 = (SCALING * qn_w[d_idx] * kn_w[d_idx]).astype(f32).reshape(128, 1)
    c2 = (SCALING * qn_b[d_idx] * kn_w[d_idx]).astype(f32).reshape(128, 1)

    lam = float(np.exp(np.sum(lambda_q1 * lambda_k1))
                - np.exp(np.sum(lambda_q2 * lambda_k2)) + LAMBDA_INIT)
    lamrow = np.tile(np.array([1.0, lam] * 4, f32), (128, 1))

    wdict = dict(w1=w1.astype(f8), w2=w2.astype(f8), wkT=wkT, g64=g64,
                 qG=qG, expand=expand, ident128b=ident128b,
                 ident64b=ident64b, c1=c1, c2=c2, lamrow=lamrow)
    WSPEC.clear()
    dtmap = {np.dtype(np.float32): F32, np.dtype(bf): BF16,
             np.dtype(f8): FP8}
    for k, v in wdict.items():
        WSPEC[k] = (v.shape, dtmap[v.dtype])
    return wdict


def pack_x(x):
    """x [N, 56, 128] f32 -> (xq8 fp8 [128, N//128, 136, 64],
    xtp bf16 [128, N//2, 128]). Both partition-major, pair q innermost
    for xq8 so DR tap pairs are stride-64 column neighbours."""
    import ml_dtypes
    f8 = ml_dtypes.float8_e4m3fn
    bf = ml_dtypes.bfloat16
    xf = np.asarray(x, np.float32)
    n = xf.shape[0]
    nt = n // 128
    x8 = xf.astype(f8)
    xq8 = np.zeros((128, nt, CC, 64), f8)
    xe = x8[0::2].reshape(nt, 64, A, E)               # [T, q, a, e]
    xo = x8[1::2].reshape(nt, 64, A, E)
    xq8[0:56, :, 3:3 + E, :] = xe.transpose(2, 0, 3, 1)
    xq8[64:120, :, 3:3 + E, :] = xo.transpose(2, 0, 3, 1)
    xb = xf.astype(bf)
    xtp = np.zeros((128, n // 2, 128), bf)
    xtp[:, :, 0:56] = xb[0::2].transpose(2, 0, 1)     # [e, P, a]
    xtp[:, :, 64:120] = xb[1::2].transpose(2, 0, 1)
    return xq8, xtp


_CACHED = {}


def kernel(**inputs):
    xq8, xtp = pack_x(inputs["x"])
    wdict = prepare_weights(
        **{k: np.asarray(v, np.float32) for k, v in inputs.items()
           if k != "x"})
    if "nc" not in _CACHED:
        _CACHED["nc"] = build_nc()
    nc = _CACHED["nc"]
    nbp = NB // 2
    in_maps = []
    for c in range(NCORES):
        m = {"x8": np.ascontiguousarray(xq8[:, c * NT:(c + 1) * NT]),
             "xt": np.ascontiguousarray(xtp[:, c * nbp:(c + 1) * nbp])}
        m.update(wdict)
        in_maps.append(m)
    res = run_bass_kernel_spmd(nc, in_maps, core_ids=list(range(NCORES)))
    return np.concatenate([np.asarray(r["out"]) for r in res.results], axis=0)


if __name__ == "__main__":
    import reference
    inputs = {k: np.asarray(v) for k, v in reference.setup_inputs().items()}
    got = kernel(**inputs)
    exp = np.asarray(reference.reference(**inputs))
    err = np.abs(got - exp).max() / np.abs(exp).max()
    print("rel err:", err)


# revision 38
# speedup vs baseline: 1.1557x; 1.1557x over previous
"""Trainium2 Bass kernel for nn_AtlasMultiDiffAttn (8-core data-parallel).

v3: interleaved pair-minor layouts (no dup-shift images), partition-major
single-DMA inputs (2 big contiguous DMAs per tile vs 12 small ones),
batched softmax phases (4x fewer act-table loads), engine rebalancing
across ACT/DVE/Pool (GPSIMD only touches SBUF -- it cannot access PSUM).

Self-contained: hardcodes shapes (x [8192,56,128] f32 -> out [8192,56] f32).
Per core: 1024 samples, 8 tiles of BT=128 samples (64 even/odd sample pairs).

Host-side packing (both partition-major, contiguous per-partition tile
lines, pair index q innermost so DoubleRow tap pairs are stride-64 column
neighbours -- no duplicated shift images):
  - xq8 fp8 [128p, NT, 136c, 64q]: p 0-55 even-sample atlas ch a, 64-119
    odd; c = e + 3 (3 zero pad each side); value x[b(q,par), a, c-3]
  - xtp bf16 [128e, npairs, 128]: cols 0-55 even-sample a, 64-119 odd

Math folds:
  - k-LN mean folded into w_k (per-head row-mean removed) => mean(k)=0
  - convs in fp8e4m3 DoubleRow, tap t = 2*jp + s where s is the DR slot
    reading column c vs c+1 (stride 64 >= 16B, satisfying DR step rule)
  - rsqrt via magic-init + 1 Newton iteration on DVE
"""
from contextlib import ExitStack

import numpy as np

import concourse.bass as bass
import concourse.tile as tile
from concourse import bacc, mybir
from concourse.bass_utils import run_bass_kernel_spmd

F32 = mybir.dt.float32
BF16 = mybir.dt.bfloat16
FP8 = mybir.dt.float8e4
I32 = mybir.dt.int32
AF = mybir.ActivationFunctionType
OP = mybir.AluOpType
AX = mybir.AxisListType
PM = mybir.MatmulPerfMode

B, A, E = 8192, 56, 128
H, HD = 4, 16
LAMBDA_INIT = 0.7
EPS = 1e-5
SCALING = HD ** -0.5
EPS_Q = float(A) ** 2 * EPS
WSCALE = 16.0

NCORES = 8
NB = B // NCORES          # 1024 samples per core
BT = 128                  # samples per tile
NT = NB // BT             # 8 tiles
NPAIR = BT // 2           # 64
CC = 136                  # padded conv1 image cols (e + 3 each side, /16)
MAGIC = 0x5F3759DF

WSPEC = {}


def _newton_rsqrt(nc, pool, v_ap, shape, tag):
    """v_ap <- rsqrt(v_ap): magic init + 1 Newton iteration (~0.2% rel)."""
    y = pool.tile(list(shape), F32, tag=f"nwy_{tag}")
    t = pool.tile(list(shape), F32, tag=f"nwt_{tag}")
    npart = v_ap.shape[0]
    ya, ta = y[0:npart], t[0:npart]
    nc.vector.tensor_scalar(out=ya.bitcast(I32), in0=v_ap.bitcast(I32),
                            scalar1=1, scalar2=None,
                            op0=OP.logical_shift_right)
    nc.vector.tensor_scalar(out=ya.bitcast(I32), in0=ya.bitcast(I32),
                            scalar1=-1, scalar2=MAGIC,
                            op0=OP.mult, op1=OP.add)
    nc.vector.tensor_tensor(out=ta, in0=ya, in1=ya, op=OP.mult)
    nc.vector.tensor_tensor(out=ta, in0=ta, in1=v_ap, op=OP.mult)
    nc.vector.tensor_scalar(out=ta, in0=ta, scalar1=-0.5, scalar2=1.5,
                            op0=OP.mult, op1=OP.add)
    nc.vector.tensor_tensor(out=v_ap, in0=ya, in1=ta, op=OP.mult)


def apx(base, offset_add, dims):
    """Raw AP with explicit free dims, keeping base's partition dim."""
    return bass.AP(tensor=base.tensor, offset=base.offset + offset_add,
                   ap=[list(base.ap[0])] + [list(d) for d in dims])


def build_tile_kernel(ctx, tc, x8_ext, xt_ext, out_ext, wext, repeat=1):
    nc = tc.nc

    consts = ctx.enter_context(tc.tile_pool(name="consts", bufs=1))
    sbA = ctx.enter_context(tc.tile_pool(name="sbA", bufs=1))
    sbH = ctx.enter_context(tc.tile_pool(name="sbH", bufs=2))
    sbX = ctx.enter_context(tc.tile_pool(name="sbX", bufs=2))
    sbK = ctx.enter_context(tc.tile_pool(name="sbK", bufs=1))
    sbQ = ctx.enter_context(tc.tile_pool(name="sbQ", bufs=2))
    sbS = ctx.enter_context(tc.tile_pool(name="sbS", bufs=1))
    sb2 = ctx.enter_context(tc.tile_pool(name="sb2", bufs=2))
    sbB = ctx.enter_context(tc.tile_pool(name="sbB", bufs=1))
    psA = ctx.enter_context(tc.tile_pool(name="psA", bufs=4, space="PSUM"))
    psK = ctx.enter_context(tc.tile_pool(name="psK", bufs=2, space="PSUM"))
    psC = ctx.enter_context(tc.tile_pool(name="psC", bufs=2, space="PSUM"))

    def cload(name):
        shape, pdt = WSPEC[name]
        t = consts.tile(list(shape), pdt, tag=f"c_{name}")
        nc.sync.dma_start(out=t[:], in_=wext[name][:])
        return t

    w1 = cload("w1")            # [128, 4, 2, 128] fp8
    w2 = cload("w2")            # [128, 4, 2, 128] fp8
    wk = cload("wkT")           # [128, 128] bf16 (head-mean folded)
    g64 = cload("g64")          # [128, 64] bf16
    qg = cload("qG")            # [128, 16] bf16
    expd = cload("expand")      # [8, 128] f32
    id128 = cload("ident128b")  # [128, 128] bf16
    idb64 = cload("ident64b")   # [64, 64] bf16
    c1 = cload("c1")            # [128, 1] f32
    c2 = cload("c2")
    lamrow = cload("lamrow")    # [128, 8] f32

    state = {}
    state1 = {}
    state2 = {}

    def phaseA1(it):
        """conv1 + transposes: PE work for tile it+? slots into the
        copy-wait seam of the previous tile's conv2."""
        xq = sbX.tile([128, CC, 64], FP8, tag="xq")
        nc.sync.dma_start(out=xq[:], in_=x8_ext[:, it])

        # ---- conv1 fp8 DoubleRow + native silu -> h8e [p,m / e,q] ----
        # jp-outer over blocks of 4 groups: consecutive matmuls share the
        # same stationary weights (weight-stationary LDW reuse)
        h8e = sbA.tile([128, 128, 64], BF16, tag="h8e")
        b0 = xq[:, 0:1, 0:1]
        for blk in range(4):
            pss = []
            for _gi in range(4):
                ps = psA.tile([128, 512], F32, tag="ps")
                pss.append(ps)
            for jp in range(4):
                for gi in range(4):
                    eb = blk * 4 + gi
                    rhs = bass.AP(
                        tensor=b0.tensor,
                        offset=b0.offset + (8 * eb + 2 * jp) * 64,
                        ap=[[list(b0.ap[0])[0], 120], [64, 2],
                            [64, 8], [1, 64]])
                    nc.tensor.matmul(pss[gi][:], w1[0:120, jp, :, :], rhs,
                                     start=(jp == 0), stop=(jp == 3),
                                     perf_mode=PM.DoubleRow)
            for gi in range(4):
                eb = blk * 4 + gi
                nc.scalar.activation(
                    h8e[:, 8 * eb:8 * eb + 8, :]
                        .rearrange("p e q -> p (e q)"),
                    pss[gi][:], AF.Silu, scale=1.0 / WSCALE)

        # ---- h transposes -> hT8 [e, m, q] (no dup; q innermost) -----
        hT8 = sbH.tile([128, 128, 64], FP8, tag="hT8")
        for g in range(8):
            psx = psC.tile([128, 8, 128], BF16, tag="psb")
            for j in range(8):
                nc.tensor.transpose(psx[:, j, :], h8e[:, :, 8 * g + j],
                                    id128[:])
            outw = apx(hT8[:, 0:1, 0:1], 8 * g, [[64, 128], [1, 8]])
            inw = apx(psx[:, 0:1, 0:1], 0, [[1, 128], [128, 8]])
            if g % 2 == 0:
                nc.vector.tensor_copy(out=outw, in_=inw)
            else:
                nc.scalar.copy(out=outw, in_=inw)

        state1[it] = hT8

    def phaseA2(it):
        """conv2 + q_acc: reads hT8 whose copies finished a phase ago."""
        hT8 = state1.pop(it)
        xt = sbX.tile([128, NPAIR, 128], BF16, tag="xt")
        nc.sync.dma_start(out=xt[:],
                          in_=xt_ext[:, it * NPAIR:(it + 1) * NPAIR, :])

        # ---- conv2 fp8 DoubleRow + silu -> h2s [p, par, a, q] --------
        # weight-stationary blocks, as in conv1
        h2s = sbA.tile([128, 2, 56, 64], BF16, tag="h2s")
        b1 = hT8[:, 0:1, 0:1]
        groups = [(ab, par) for ab in range(7) for par in range(2)]
        for blk in range(0, 14, 4):
            gblk = groups[blk:blk + 4]
            pss = []
            for _gi in gblk:
                ps = psA.tile([128, 512], F32, tag="ps")
                pss.append(ps)
            for jp in range(4):
                for gi, (ab, par) in enumerate(gblk):
                    rhs = bass.AP(
                        tensor=b1.tensor,
                        offset=b1.offset
                        + (8 * ab + 2 * jp + 64 * par) * 64,
                        ap=[[list(b1.ap[0])[0], 128], [64, 2],
                            [64, 8], [1, 64]])
                    nc.tensor.matmul(pss[gi][:], w2[:, jp, :, :], rhs,
                                     start=(jp == 0), stop=(jp == 3),
                                     perf_mode=PM.DoubleRow)
            for gi, (ab, par) in enumerate(gblk):
                nc.scalar.activation(
                    h2s[:, par, 8 * ab:8 * ab + 8, :]
                        .rearrange("p a q -> p (a q)"),
                    pss[gi][:], AF.Silu, scale=1.0 / WSCALE)

        # ---- q_acc[p, n=2q+par] = sum_a h2s (split for engine balance)
        q_acc = sbQ.tile([128, 128], BF16, tag="qacc")
        rha = sbQ.tile([128, 2, 2, 64], BF16, tag="rha")
        with nc.allow_low_precision(reason="56-term sum; LN follows"):
            for par in range(2):
                for half in range(2):
                    a0 = 28 * half
                    eng = nc.vector
                    eng.reduce_sum(
                        rha[:, par, half, :],
                        h2s[:, par, a0:a0 + 28, :].transpose([0, 2, 1]),
                        axis=AX.X)
            for par in range(2):
                nc.vector.tensor_tensor(
                    out=apx(q_acc[:, 0:1], par, [[2, 64]]),
                    in0=rha[:, par, 0, :], in1=rha[:, par, 1, :],
                    op=OP.add)

        state[it] = (xt, q_acc)

    def phaseB1(it):
        xt, q_acc = state.pop(it)
        # ---- q-side stats/LN (psum segments: muq 0:128 then reused for
        # expd-vq; sumq2 128:256; expd-mu 256:384) ---------------------
        qpsA = psK.tile([128, 448], F32, tag="pk")
        nc.tensor.matmul(qpsA[0:8, 0:128], qg[:, 0:8], q_acc[:],
                         start=True, stop=True)                 # muq
        q2 = sbQ.tile([128, 128], BF16, tag="q2")
        nc.vector.tensor_tensor(out=q2[:], in0=q_acc[:], in1=q_acc[:],
                                op=OP.mult)
        nc.tensor.matmul(qpsA[0:8, 128:256], qg[:, 8:16], q2[:],
                         start=True, stop=True)                 # sum q^2
        muq = sbQ.tile([128, 128], F32, tag="muq")
        nc.vector.tensor_copy(out=muq[0:8, :], in_=qpsA[0:8, 0:128])
        vq = sbQ.tile([128, 128], F32, tag="vq")
        nc.vector.tensor_tensor(out=vq[0:8, :], in0=muq[0:8, :],
                                in1=muq[0:8, :], op=OP.mult)
        nc.vector.scalar_tensor_tensor(
            out=vq[0:8, :], in0=qpsA[0:8, 128:256], scalar=1.0 / HD,
            in1=vq[0:8, :], op0=OP.mult, op1=OP.subtract)
        nc.vector.tensor_scalar_add(vq[0:8, :], vq[0:8, :], EPS_Q)
        _newton_rsqrt(nc, sbQ, vq[0:8, :], [128, 128], "rq")
        nc.tensor.matmul(qpsA[:, 256:384], expd[:], muq[0:8, :],
                         start=True, stop=True)
        nc.tensor.matmul(qpsA[:, 0:128], expd[:], vq[0:8, :],
                         start=True, stop=True)
        qhat = sbQ.tile([128, 128], F32, tag="qhat")
        nc.vector.tensor_tensor(out=qhat[:], in0=q_acc[:],
                                in1=qpsA[:, 256:384], op=OP.subtract)
        nc.vector.tensor_tensor(out=qhat[:], in0=qhat[:],
                                in1=qpsA[:, 0:128], op=OP.mult)
        qb = sbQ.tile([128, 128], F32, tag="qb")
        nc.vector.tensor_scalar(out=qb[:], in0=qhat[:], scalar1=c1[:],
                                scalar2=c2[:], op0=OP.mult, op1=OP.add)
        qbb = sbQ.tile([128, 128], BF16, tag="qbb")
        nc.gpsimd.tensor_copy(out=qbb[:], in_=qb[:])

        # ---- k projection chunks -> k_T SBUF; k2/qbk mostly on Pool --
        k_T = sbK.tile([128, NPAIR, 112], BF16, tag="kT")
        k2 = sbK.tile([128, NPAIR, 112], BF16, tag="k2")
        qbk = sbK.tile([128, NPAIR, 2, A], BF16, tag="qbk")
        for ch in range(16):
            q0 = ch * 4
            pk = psK.tile([128, 448], F32, tag="pk")
            krhs = apx(xt[:, 0:1, 0:1], q0 * 128,
                       [[128, 4], [64, 2], [1, A]])
            nc.tensor.matmul(pk[:, 0:448], wk[:], krhs, start=True,
                             stop=True)
            kslice = k_T[:, q0:q0 + 4, :].rearrange("p q r -> p (q r)")
            if ch % 2 == 0:
                nc.vector.tensor_copy(out=kslice, in_=pk[:, 0:448])
            else:
                nc.scalar.copy(out=kslice, in_=pk[:, 0:448])
            k2o = k2[:, q0:q0 + 4, :].rearrange("p q r -> p (q r)")
            ksb = k_T[:, q0:q0 + 4, :].rearrange("p q r -> p (q r)")
            k2e = nc.vector if ch % 4 == 0 else nc.gpsimd
            k2e.tensor_tensor(out=k2o, in0=ksb, in1=ksb, op=OP.mult)
            qbe = nc.vector if ch % 4 == 2 else nc.gpsimd
            qbe.tensor_tensor(
                out=qbk[:, q0:q0 + 4, :, :],
                in0=k_T[:, q0:q0 + 4, :].rearrange("p q (c a) -> p q c a",
                                                   c=2),
                in1=qbb[:, 2 * q0:2 * q0 + 8]
                    .rearrange("p (q c) -> p q c", c=2).unsqueeze(3)
                    .to_broadcast((128, 4, 2, A)),
                op=OP.mult)

        # ---- k stats: 2 col-tiled matmuls per 4-pair group -----------
        stats_sb = sb2.tile([64, NPAIR, 112], BF16, tag="statsb")
        st_engs = ([nc.vector] * 3 + [nc.scalar] * 1) * 4
        for cki in range(16):
            p0 = cki * 4
            pst = psK.tile([128, 448], F32, tag="pk")
            nc.tensor.matmul(pst[0:32, 0:448], g64[:, 0:32],
                             k2[:, p0:p0 + 4, :], start=True, stop=True)
            nc.tensor.matmul(
                pst[32:64, 0:448], g64[:, 32:64],
                qbk[:, p0:p0 + 4, :, :].rearrange("p q c a -> p q (c a)"),
                start=True, stop=True, tile_position=(0, 32))
            so = stats_sb[0:64, p0:p0 + 4, :].rearrange("p q r -> p (q r)")
            eng = st_engs[cki]
            if eng is nc.scalar:
                nc.scalar.copy(out=so, in_=pst[0:64, 0:448])
            else:
                eng.tensor_copy(out=so, in_=pst[0:64, 0:448])

        # ---- per-l transposes -> statsBall[:, it] [128=n, 56, 2, 8] --
        statsBall = state2["all"]
        svb = stats_sb[:].rearrange("p q (c l) -> p (q c) l", c=2)
        for li in range(4):
            l0 = li * 14
            pstb = psC.tile([128, 14, 64], BF16, tag="psb")
            for j in range(14):
                nc.tensor.transpose(pstb[:, j, :], svb[0:64, :, l0 + j],
                                    idb64[:])
            inw = apx(pstb[:, 0:1, 0:1], 0, [[64, 14], [32, 2], [1, 8]])
            if li % 2 == 0:
                nc.vector.tensor_copy(
                    out=statsBall[:, it, l0:l0 + 14, :, :], in_=inw)
            else:
                nc.scalar.copy(out=statsBall[:, it, l0:l0 + 14, :, :],
                               in_=inw)

    def phaseB2quad(t0):
        # score assembly + double softmax for 4 tiles in one pass
        sB = state2["all"]
        sk2 = sB[:, t0:t0 + 4, :, 0, :]              # [128, 4, 56, 8]
        QK = sB[:, t0:t0 + 4, :, 1, :]
        vk = sbB.tile([128, 4, A, 8], F32, tag="vk")
        nc.vector.tensor_scalar(out=vk[:], in0=sk2, scalar1=1.0 / HD,
                                scalar2=EPS, op0=OP.mult, op1=OP.add)
        _newton_rsqrt(nc, sbB, vk[:, 0:2], [128, 2, A, 8], "rk")
        _newton_rsqrt(nc, sbB, vk[:, 2:4], [128, 2, A, 8], "rk")
        # s_sc reuses vk in place: vk <- QK * vk
        nc.vector.tensor_tensor(out=vk[:], in0=QK, in1=vk[:], op=OP.mult)
        s_sc = vk
        # scores are bounded (Cauchy-Schwarz): exp cannot overflow.
        nc.scalar.activation(s_sc[:], s_sc[:], AF.Exp)
        z1 = sbB.tile([128, 4, 8], F32, tag="z1")
        nc.vector.reduce_sum(z1[:], s_sc[:].transpose([0, 1, 3, 2]),
                             axis=AX.X)
        rz1 = sbB.tile([128, 4, 8], F32, tag="rz1")
        nc.vector.reciprocal(rz1[:], z1[:])
        nc.vector.tensor_tensor(
            out=rz1[:], in0=rz1[:],
            in1=lamrow[:].unsqueeze(1).to_broadcast((128, 4, 8)),
            op=OP.mult)
        nc.vector.tensor_tensor(
            out=s_sc[:], in0=s_sc[:],
            in1=rz1[:].unsqueeze(2).to_broadcast((128, 4, A, 8)),
            op=OP.mult)
        dd = sbB.tile([128, 4, A, 4], F32, tag="dd")
        nc.vector.tensor_tensor(out=dd[:], in0=s_sc[:, :, :, 0:8:2],
                                in1=s_sc[:, :, :, 1:8:2], op=OP.subtract)
        nc.scalar.activation(dd[:], dd[:], AF.Exp)
        z2 = sbB.tile([128, 4, 4], F32, tag="z2")
        nc.vector.reduce_sum(z2[:], dd[:].transpose([0, 1, 3, 2]),
                             axis=AX.X)
        rz2 = sbB.tile([128, 4, 4], F32, tag="rz2")
        nc.vector.reciprocal(rz2[:], z2[:])
        nc.vector.tensor_scalar_mul(rz2[:], rz2[:], 1.0 / H)
        nc.vector.tensor_tensor(
            out=dd[:], in0=dd[:],
            in1=rz2[:].unsqueeze(2).to_broadcast((128, 4, A, 4)),
            op=OP.mult)
        ot = sbB.tile([128, 4, A], F32, tag="ot")
        nc.vector.reduce_sum(ot[:], dd[:], axis=AX.X)

        outv = out_ext[t0 * BT:(t0 + 4) * BT, :].rearrange(
            "(t n) a -> n t a", t=4)
        nc.sync.dma_start(out=outv, in_=ot[:])

    for _rep in range(repeat):
        phaseA1(0)
        phaseA1(1)
        phaseA2(0)
        pend = []
        for it in range(NT):
            if it + 2 < NT:
                phaseA1(it + 2)
            if it + 1 < NT:
                phaseA2(it + 1)
            phaseB1(it)
            pend.append(it)
            if len(pend) == 4:
                for t in pend:
                    phaseB2(t)
                pend = []


def build_nc(repeat=1):
    nc = bacc.Bacc(target_bir_lowering=False, trn_type="TRN2")
    x8_ext = nc.declare_dram_parameter("x8", [128, NT, CC, 64], FP8,
                                       isOutput=False)
    xt_ext = nc.declare_dram_parameter("xt", [128, NB // 2, 128], BF16,
                                       isOutput=False)
    out_ext = nc.declare_dram_parameter("out", [NB, A], F32, isOutput=True)
    wext = {}
    for name, (shape, dt) in WSPEC.items():
        wext[name] = nc.declare_dram_parameter(name, list(shape), dt,
                                               isOutput=False)
    with tile.TileContext(nc) as tc:
        with ExitStack() as ctx:
            build_tile_kernel(ctx, tc, x8_ext, xt_ext, out_ext, wext,
                              repeat=repeat)
    nc.compile()
    return nc


def prepare_weights(w_emb, b_emb, w_atlas, b_atlas, w_k, qn_w, qn_b, kn_w,
                    kn_b, lambda_q1, lambda_k1, lambda_q2, lambda_k2):
    import ml_dtypes
    bf = ml_dtypes.bfloat16
    f8 = ml_dtypes.float8_e4m3fn
    f32 = np.float32

    assert np.allclose(b_atlas, 0.0), "kernel assumes b_atlas == 0"
    assert np.allclose(b_emb, 0.0), "kernel assumes b_emb == 0"
    assert np.allclose(kn_b, 0.0), "kernel assumes kn_b == 0"

    # conv1 lhsT [120, 4jp, 2s, 128]: tap 2jp+s; even block cols 3:59,
    # odd block cols 67:123 (the m output partition layout)
    w1 = np.zeros((128, 4, 2, 128), f32)
    w2 = np.zeros((128, 4, 2, 128), f32)
    for t in range(7):
        jp, s = t // 2, t % 2
        blk = np.transpose(w_emb[:, :, t]).astype(f32) * WSCALE
        w1[0:56, jp, s, 3:59] = blk
        w1[64:120, jp, s, 67:123] = blk
        w2[:, jp, s, :] = np.transpose(w_atlas[:, :, t]).astype(f32) * WSCALE

    # k projection with per-head row mean folded out (=> mean_h(k) == 0)
    wkf = np.asarray(w_k, f32)
    wkp = wkf - wkf.reshape(2 * H, HD, E).mean(axis=1, keepdims=True).repeat(
        HD, axis=1).reshape(E, E)
    wkT = np.ascontiguousarray(np.transpose(wkp)).astype(bf)

    G = np.zeros((128, 8), f32)
    for o in range(128):
        G[o, o // HD] = 1.0
    Z24 = np.zeros((128, 24), f32)
    g64 = np.ascontiguousarray(np.concatenate(
        [G, Z24, G, Z24], axis=1)).astype(bf)

    qG = np.ascontiguousarray(
        np.concatenate([G / HD, G], axis=1)).astype(bf)

    expand = np.ascontiguousarray(G.T).astype(f32)            # [8, 128]
    ident128b = np.eye(128, dtype=bf)
    ident64b = np.eye(64, dtype=bf)

    d_idx = np.arange(E) % HD
    c1 = (SCALING * qn_w[d_idx] * kn_w[d_idx]).astype(f32).reshape(128, 1)
    c2 = (SCALING * qn_b[d_idx] * kn_w[d_idx]).astype(f32).reshape(128, 1)

    lam = float(np.exp(np.sum(lambda_q1 * lambda_k1))
                - np.exp(np.sum(lambda_q2 * lambda_k2)) + LAMBDA_INIT)
    lamrow = np.tile(np.array([1.0, lam] * 4, f32), (128, 1))

    wdict = dict(w1=w1.astype(f8), w2=w2.astype(f8), wkT=wkT, g64=g64,
                 qG=qG, expand=expand, ident128b=ident128b,
                 ident64b=ident64b, c1=c1, c2=c2, lamrow=lamrow)
    WSPEC.clear()
    dtmap = {np.dtype(np.float32): F32, np.dtype(bf): BF16,
             np.dtype(f8): FP8}
    for k, v in wdict.items():
        WSPEC[k] = (v.shape, dtmap[v.dtype])
    return wdict


def pack_x(x):
    """x [N, 56, 128] f32 -> (xq8 fp8 [128, N//128, 136, 64],
    xtp bf16 [128, N//2, 128]). Both partition-major, pair q innermost
    for xq8 so DR tap pairs are stride-64 column neighbours."""
    import ml_dtypes
    f8 = ml_dtypes.float8_e4m3fn
    bf = ml_dtypes.bfloat16
    xf = np.asarray(x, np.float32)
    n = xf.shape[0]
    nt = n // 128
    x8 = xf.astype(f8)
    xq8 = np.zeros((128, nt, CC, 64), f8)
    xe = x8[0::2].reshape(nt, 64, A, E)               # [T, q, a, e]
    xo = x8[1::2].reshape(nt, 64, A, E)
    xq8[0:56, :, 3:3 + E, :] = xe.transpose(2, 0, 3, 1)
    xq8[64:120, :, 3:3 + E, :] = xo.transpose(2, 0, 3, 1)
    xb = xf.astype(bf)
    xtp = np.zeros((128, n // 2, 128), bf)
    xtp[:, :, 0:56] = xb[0::2].transpose(2, 0, 1)     # [e, P, a]
    xtp[:, :, 64:120] = xb[1::2].transpose(2, 0, 1)
    return xq8, xtp


_CACHED = {}


def kernel(**inputs):
    xq8, xtp = pack_x(inputs["x"])
    wdict = prepare_weights(
        **{k: np.asarray(v, np.float32) for k, v in inputs.items()
           if k != "x"})
    if "nc" not in _CACHED:
        _CACHED["nc"] = build_nc()
    nc = _CACHED["nc"]
    nbp = NB // 2
    in_maps = []
    for c in range(NCORES):
        m = {"x8": np.ascontiguousarray(xq8[:, c * NT:(c + 1) * NT]),
             "xt": np.ascontiguousarray(xtp[:, c * nbp:(c + 1) * nbp])}
        m.update(wdict)
        in_maps.append(m)
    res = run_bass_kernel_spmd(nc, in_maps, core_ids=list(range(NCORES)))
    return np.concatenate([np.asarray(r["out"]) for r in res.results], axis=0)


if __name__ == "__main__":
    import reference
    inputs = {k: np.asarray(v) for k, v in reference.setup_inputs().items()}
    got = kernel(**inputs)
    exp = np.asarray(reference.reference(**inputs))
    err = np.abs(got - exp).max() / np.abs(exp).max()
    print("rel err:", err)


# revision 40
# speedup vs baseline: 1.1651x; 1.0082x over previous
"""Trainium2 Bass kernel for nn_AtlasMultiDiffAttn (8-core data-parallel).

v3: interleaved pair-minor layouts (no dup-shift images), partition-major
single-DMA inputs (2 big contiguous DMAs per tile vs 12 small ones),
batched softmax phases (4x fewer act-table loads), engine rebalancing
across ACT/DVE/Pool (GPSIMD only touches SBUF -- it cannot access PSUM).

Self-contained: hardcodes shapes (x [8192,56,128] f32 -> out [8192,56] f32).
Per core: 1024 samples, 8 tiles of BT=128 samples (64 even/odd sample pairs).

Host-side packing (both partition-major, contiguous per-partition tile
lines, pair index q innermost so DoubleRow tap pairs are stride-64 column
neighbours -- no duplicated shift images):
  - xq8 fp8 [128p, NT, 136c, 64q]: p 0-55 even-sample atlas ch a, 64-119
    odd; c = e + 3 (3 zero pad each side); value x[b(q,par), a, c-3]
  - xtp bf16 [128e, npairs, 128]: cols 0-55 even-sample a, 64-119 odd

Math folds:
  - k-LN mean folded into w_k (per-head row-mean removed) => mean(k)=0
  - convs in fp8e4m3 DoubleRow, tap t = 2*jp + s where s is the DR slot
    reading column c vs c+1 (stride 64 >= 16B, satisfying DR step rule)
  - rsqrt via magic-init + 1 Newton iteration on DVE
"""
from contextlib import ExitStack

import numpy as np

import concourse.bass as bass
import concourse.tile as tile
from concourse import bacc, mybir
from concourse.bass_utils import run_bass_kernel_spmd

F32 = mybir.dt.float32
BF16 = mybir.dt.bfloat16
FP8 = mybir.dt.float8e4
I32 = mybir.dt.int32
AF = mybir.ActivationFunctionType
OP = mybir.AluOpType
AX = mybir.AxisListType
PM = mybir.MatmulPerfMode

B, A, E = 8192, 56, 128
H, HD = 4, 16
LAMBDA_INIT = 0.7
EPS = 1e-5
SCALING = HD ** -0.5
EPS_Q = float(A) ** 2 * EPS
WSCALE = 16.0

NCORES = 8
NB = B // NCORES          # 1024 samples per core
BT = 128                  # samples per tile
NT = NB // BT             # 8 tiles
NPAIR = BT // 2           # 64
CC = 136                  # padded conv1 image cols (e + 3 each side, /16)
MAGIC = 0x5F3759DF

WSPEC = {}


def _newton_rsqrt(nc, pool, v_ap, shape, tag):
    """v_ap <- rsqrt(v_ap): magic init + 1 Newton iteration (~0.2% rel)."""
    y = pool.tile(list(shape), F32, tag=f"nwy_{tag}")
    t = pool.tile(list(shape), F32, tag=f"nwt_{tag}")
    npart = v_ap.shape[0]
    ya, ta = y[0:npart], t[0:npart]
    nc.vector.tensor_scalar(out=ya.bitcast(I32), in0=v_ap.bitcast(I32),
                            scalar1=1, scalar2=None,
                            op0=OP.logical_shift_right)
    nc.vector.tensor_scalar(out=ya.bitcast(I32), in0=ya.bitcast(I32),
                            scalar1=-1, scalar2=MAGIC,
                            op0=OP.mult, op1=OP.add)
    nc.vector.tensor_tensor(out=ta, in0=ya, in1=ya, op=OP.mult)
    nc.vector.tensor_tensor(out=ta, in0=ta, in1=v_ap, op=OP.mult)
    nc.vector.tensor_scalar(out=ta, in0=ta, scalar1=-0.5, scalar2=1.5,
                            op0=OP.mult, op1=OP.add)
    nc.vector.tensor_tensor(out=v_ap, in0=ya, in1=ta, op=OP.mult)


def apx(base, offset_add, dims):
    """Raw AP with explicit free dims, keeping base's partition dim."""
    return bass.AP(tensor=base.tensor, offset=base.offset + offset_add,
                   ap=[list(base.ap[0])] + [list(d) for d in dims])


def build_tile_kernel(ctx, tc, x8_ext, xt_ext, out_ext, wext, repeat=1):
    nc = tc.nc

    consts = ctx.enter_context(tc.tile_pool(name="consts", bufs=1))
    sbA = ctx.enter_context(tc.tile_pool(name="sbA", bufs=1))
    sbH = ctx.enter_context(tc.tile_pool(name="sbH", bufs=2))
    sbX = ctx.enter_context(tc.tile_pool(name="sbX", bufs=2))
    sbK = ctx.enter_context(tc.tile_pool(name="sbK", bufs=1))
    sbQ = ctx.enter_context(tc.tile_pool(name="sbQ", bufs=2))
    sbS = ctx.enter_context(tc.tile_pool(name="sbS", bufs=1))
    sb2 = ctx.enter_context(tc.tile_pool(name="sb2", bufs=2))
    sbB = ctx.enter_context(tc.tile_pool(name="sbB", bufs=1))
    psA = ctx.enter_context(tc.tile_pool(name="psA", bufs=4, space="PSUM"))
    psK = ctx.enter_context(tc.tile_pool(name="psK", bufs=2, space="PSUM"))
    psC = ctx.enter_context(tc.tile_pool(name="psC", bufs=2, space="PSUM"))

    def cload(name):
        shape, pdt = WSPEC[name]
        t = consts.tile(list(shape), pdt, tag=f"c_{name}")
        nc.sync.dma_start(out=t[:], in_=wext[name][:])
        return t

    w1 = cload("w1")            # [128, 4, 2, 128] fp8
    w2 = cload("w2")            # [128, 4, 2, 128] fp8
    wk = cload("wkT")           # [128, 128] bf16 (head-mean folded)
    g64 = cload("g64")          # [128, 64] bf16
    qg = cload("qG")            # [128, 16] bf16
    expd = cload("expand")      # [8, 128] f32
    id128 = cload("ident128b")  # [128, 128] bf16
    idb64 = cload("ident64b")   # [64, 64] bf16
    c1 = cload("c1")            # [128, 1] f32
    c2 = cload("c2")
    lamrow = cload("lamrow")    # [128, 8] f32

    state = {}
    state1 = {}
    state2 = {}

    def phaseA1(it):
        """conv1 + transposes: PE work for tile it+? slots into the
        copy-wait seam of the previous tile's conv2."""
        xq = sbX.tile([128, CC, 64], FP8, tag="xq")
        nc.sync.dma_start(out=xq[:], in_=x8_ext[:, it])

        # ---- conv1 fp8 DoubleRow + native silu -> h8e [p,m / e,q] ----
        # jp-outer over blocks of 4 groups: consecutive matmuls share the
        # same stationary weights (weight-stationary LDW reuse)
        h8e = sbA.tile([128, 128, 64], BF16, tag="h8e")
        b0 = xq[:, 0:1, 0:1]
        for blk in range(4):
            pss = []
            for _gi in range(4):
                ps = psA.tile([128, 512], F32, tag="ps")
                pss.append(ps)
            for jp in range(4):
                for gi in range(4):
                    eb = blk * 4 + gi
                    rhs = bass.AP(
                        tensor=b0.tensor,
                        offset=b0.offset + (8 * eb + 2 * jp) * 64,
                        ap=[[list(b0.ap[0])[0], 120], [64, 2],
                            [64, 8], [1, 64]])
                    nc.tensor.matmul(pss[gi][:], w1[0:120, jp, :, :], rhs,
                                     start=(jp == 0), stop=(jp == 3),
                                     perf_mode=PM.DoubleRow)
            for gi in range(4):
                eb = blk * 4 + gi
                nc.scalar.activation(
                    h8e[:, 8 * eb:8 * eb + 8, :]
                        .rearrange("p e q -> p (e q)"),
                    pss[gi][:], AF.Silu, scale=1.0 / WSCALE)

        # ---- h transposes -> hT8 [e, m, q] (no dup; q innermost) -----
        hT8 = sbH.tile([128, 128, 64], FP8, tag="hT8")
        for g in range(8):
            psx = psC.tile([128, 8, 128], BF16, tag="psb")
            for j in range(8):
                nc.tensor.transpose(psx[:, j, :], h8e[:, :, 8 * g + j],
                                    id128[:])
            outw = apx(hT8[:, 0:1, 0:1], 8 * g, [[64, 128], [1, 8]])
            inw = apx(psx[:, 0:1, 0:1], 0, [[1, 128], [128, 8]])
            if g % 2 == 0:
                nc.vector.tensor_copy(out=outw, in_=inw)
            else:
                nc.scalar.copy(out=outw, in_=inw)

        state1[it] = hT8

    def phaseA2(it):
        """conv2 + q_acc: reads hT8 whose copies finished a phase ago."""
        hT8 = state1.pop(it)
        xt = sbX.tile([128, NPAIR, 128], BF16, tag="xt")
        nc.sync.dma_start(out=xt[:],
                          in_=xt_ext[:, it * NPAIR:(it + 1) * NPAIR, :])

        # ---- conv2 fp8 DoubleRow + silu -> h2s [p, par, a, q] --------
        # weight-stationary blocks, as in conv1
        h2s = sbA.tile([128, 2, 56, 64], BF16, tag="h2s")
        b1 = hT8[:, 0:1, 0:1]
        groups = [(ab, par) for ab in range(7) for par in range(2)]
        for blk in range(0, 14, 4):
            gblk = groups[blk:blk + 4]
            pss = []
            for _gi in gblk:
                ps = psA.tile([128, 512], F32, tag="ps")
                pss.append(ps)
            for jp in range(4):
                for gi, (ab, par) in enumerate(gblk):
                    rhs = bass.AP(
                        tensor=b1.tensor,
                        offset=b1.offset
                        + (8 * ab + 2 * jp + 64 * par) * 64,
                        ap=[[list(b1.ap[0])[0], 128], [64, 2],
                            [64, 8], [1, 64]])
                    nc.tensor.matmul(pss[gi][:], w2[:, jp, :, :], rhs,
                                     start=(jp == 0), stop=(jp == 3),
                                     perf_mode=PM.DoubleRow)
            for gi, (ab, par) in enumerate(gblk):
                nc.scalar.activation(
                    h2s[:, par, 8 * ab:8 * ab + 8, :]
                        .rearrange("p a q -> p (a q)"),
                    pss[gi][:], AF.Silu, scale=1.0 / WSCALE)

        # ---- q_acc[p, n=2q+par] = sum_a h2s (split for engine balance)
        q_acc = sbQ.tile([128, 128], BF16, tag="qacc")
        rha = sbQ.tile([128, 2, 2, 64], BF16, tag="rha")
        with nc.allow_low_precision(reason="56-term sum; LN follows"):
            for par in range(2):
                for half in range(2):
                    a0 = 28 * half
                    eng = nc.vector
                    eng.reduce_sum(
                        rha[:, par, half, :],
                        h2s[:, par, a0:a0 + 28, :].transpose([0, 2, 1]),
                        axis=AX.X)
            for par in range(2):
                nc.vector.tensor_tensor(
                    out=apx(q_acc[:, 0:1], par, [[2, 64]]),
                    in0=rha[:, par, 0, :], in1=rha[:, par, 1, :],
                    op=OP.add)

        state[it] = (xt, q_acc)

    def phaseB1(it):
        xt, q_acc = state.pop(it)
        # ---- q-side stats/LN (psum segments: muq 0:128 then reused for
        # expd-vq; sumq2 128:256; expd-mu 256:384) ---------------------
        qpsA = psK.tile([128, 448], F32, tag="pk")
        nc.tensor.matmul(qpsA[0:8, 0:128], qg[:, 0:8], q_acc[:],
                         start=True, stop=True)                 # muq
        q2 = sbQ.tile([128, 128], BF16, tag="q2")
        nc.vector.tensor_tensor(out=q2[:], in0=q_acc[:], in1=q_acc[:],
                                op=OP.mult)
        nc.tensor.matmul(qpsA[0:8, 128:256], qg[:, 8:16], q2[:],
                         start=True, stop=True)                 # sum q^2
        muq = sbQ.tile([128, 128], F32, tag="muq")
        nc.vector.tensor_copy(out=muq[0:8, :], in_=qpsA[0:8, 0:128])
        vq = sbQ.tile([128, 128], F32, tag="vq")
        nc.vector.tensor_tensor(out=vq[0:8, :], in0=muq[0:8, :],
                                in1=muq[0:8, :], op=OP.mult)
        nc.vector.scalar_tensor_tensor(
            out=vq[0:8, :], in0=qpsA[0:8, 128:256], scalar=1.0 / HD,
            in1=vq[0:8, :], op0=OP.mult, op1=OP.subtract)
        nc.vector.tensor_scalar_add(vq[0:8, :], vq[0:8, :], EPS_Q)
        _newton_rsqrt(nc, sbQ, vq[0:8, :], [128, 128], "rq")
        nc.tensor.matmul(qpsA[:, 256:384], expd[:], muq[0:8, :],
                         start=True, stop=True)
        nc.tensor.matmul(qpsA[:, 0:128], expd[:], vq[0:8, :],
                         start=True, stop=True)
        qhat = sbQ.tile([128, 128], F32, tag="qhat")
        nc.vector.tensor_tensor(out=qhat[:], in0=q_acc[:],
                                in1=qpsA[:, 256:384], op=OP.subtract)
        nc.vector.tensor_tensor(out=qhat[:], in0=qhat[:],
                                in1=qpsA[:, 0:128], op=OP.mult)
        qb = sbQ.tile([128, 128], F32, tag="qb")
        nc.vector.tensor_scalar(out=qb[:], in0=qhat[:], scalar1=c1[:],
                                scalar2=c2[:], op0=OP.mult, op1=OP.add)
        qbb = sbQ.tile([128, 128], BF16, tag="qbb")
        nc.gpsimd.tensor_copy(out=qbb[:], in_=qb[:])

        # ---- k projection chunks -> k_T SBUF; k2/qbk mostly on Pool --
        k_T = sbK.tile([128, NPAIR, 112], BF16, tag="kT")
        k2 = sbK.tile([128, NPAIR, 112], BF16, tag="k2")
        qbk = sbK.tile([128, NPAIR, 2, A], BF16, tag="qbk")
        for ch in range(16):
            q0 = ch * 4
            pk = psK.tile([128, 448], F32, tag="pk")
            krhs = apx(xt[:, 0:1, 0:1], q0 * 128,
                       [[128, 4], [64, 2], [1, A]])
            nc.tensor.matmul(pk[:, 0:448], wk[:], krhs, start=True,
                             stop=True)
            kslice = k_T[:, q0:q0 + 4, :].rearrange("p q r -> p (q r)")
            if ch % 2 == 0:
                nc.vector.tensor_copy(out=kslice, in_=pk[:, 0:448])
            else:
                nc.scalar.copy(out=kslice, in_=pk[:, 0:448])
            k2o = k2[:, q0:q0 + 4, :].rearrange("p q r -> p (q r)")
            ksb = k_T[:, q0:q0 + 4, :].rearrange("p q r -> p (q r)")
            k2e = nc.vector if ch % 4 == 0 else nc.gpsimd
            k2e.tensor_tensor(out=k2o, in0=ksb, in1=ksb, op=OP.mult)
            qbe = nc.vector if ch % 4 == 2 else nc.gpsimd
            qbe.tensor_tensor(
                out=qbk[:, q0:q0 + 4, :, :],
                in0=k_T[:, q0:q0 + 4, :].rearrange("p q (c a) -> p q c a",
                                                   c=2),
                in1=qbb[:, 2 * q0:2 * q0 + 8]
                    .rearrange("p (q c) -> p q c", c=2).unsqueeze(3)
                    .to_broadcast((128, 4, 2, A)),
                op=OP.mult)

        # ---- k stats: 2 col-tiled matmuls per 4-pair group -----------
        stats_sb = sb2.tile([64, NPAIR, 112], BF16, tag="statsb")
        st_engs = ([nc.vector] * 3 + [nc.scalar] * 1) * 4
        for cki in range(16):
            p0 = cki * 4
            pst = psK.tile([128, 448], F32, tag="pk")
            nc.tensor.matmul(pst[0:32, 0:448], g64[:, 0:32],
                             k2[:, p0:p0 + 4, :], start=True, stop=True)
            nc.tensor.matmul(
                pst[32:64, 0:448], g64[:, 32:64],
                qbk[:, p0:p0 + 4, :, :].rearrange("p q c a -> p q (c a)"),
                start=True, stop=True, tile_position=(0, 32))
            so = stats_sb[0:64, p0:p0 + 4, :].rearrange("p q r -> p (q r)")
            eng = st_engs[cki]
            if eng is nc.scalar:
                nc.scalar.copy(out=so, in_=pst[0:64, 0:448])
            else:
                eng.tensor_copy(out=so, in_=pst[0:64, 0:448])

        # ---- per-l transposes -> statsBall[:, it] [128=n, 56, 2, 8] --
        statsBall = state2["all"]
        svb = stats_sb[:].rearrange("p q (c l) -> p (q c) l", c=2)
        for li in range(4):
            l0 = li * 14
            pstb = psC.tile([128, 14, 64], BF16, tag="psb")
            for j in range(14):
                nc.tensor.transpose(pstb[:, j, :], svb[0:64, :, l0 + j],
                                    idb64[:])
            inw = apx(pstb[:, 0:1, 0:1], 0, [[64, 14], [32, 2], [1, 8]])
            if li % 2 == 0:
                nc.vector.tensor_copy(
                    out=statsBall[:, it, l0:l0 + 14, :, :], in_=inw)
            else:
                nc.scalar.copy(out=statsBall[:, it, l0:l0 + 14, :, :],
                               in_=inw)

    def phaseB2quad(t0):
        # score assembly + double softmax for 4 tiles in one pass
        sB = state2["all"]
        sk2 = sB[:, t0:t0 + 4, :, 0, :]              # [128, 4, 56, 8]
        QK = sB[:, t0:t0 + 4, :, 1, :]
        vk = sbB.tile([128, 4, A, 8], F32, tag="vk")
        nc.vector.tensor_scalar(out=vk[:], in0=sk2, scalar1=1.0 / HD,
                                scalar2=EPS, op0=OP.mult, op1=OP.add)
        _newton_rsqrt(nc, sbB, vk[:, 0:2], [128, 2, A, 8], "rk")
        _newton_rsqrt(nc, sbB, vk[:, 2:4], [128, 2, A, 8], "rk")
        # s_sc reuses vk in place: vk <- QK * vk
        nc.vector.tensor_tensor(out=vk[:], in0=QK, in1=vk[:], op=OP.mult)
        s_sc = vk
        # scores are bounded (Cauchy-Schwarz): exp cannot overflow.
        nc.scalar.activation(s_sc[:], s_sc[:], AF.Exp)
        z1 = sbB.tile([128, 4, 8], F32, tag="z1")
        nc.vector.reduce_sum(z1[:], s_sc[:].transpose([0, 1, 3, 2]),
                             axis=AX.X)
        rz1 = sbB.tile([128, 4, 8], F32, tag="rz1")
        nc.vector.reciprocal(rz1[:], z1[:])
        nc.vector.tensor_tensor(
            out=rz1[:], in0=rz1[:],
            in1=lamrow[:].unsqueeze(1).to_broadcast((128, 4, 8)),
            op=OP.mult)
        nc.vector.tensor_tensor(
            out=s_sc[:], in0=s_sc[:],
            in1=rz1[:].unsqueeze(2).to_broadcast((128, 4, A, 8)),
            op=OP.mult)
        dd = sbB.tile([128, 4, A, 4], F32, tag="dd")
        nc.vector.tensor_tensor(out=dd[:], in0=s_sc[:, :, :, 0:8:2],
                                in1=s_sc[:, :, :, 1:8:2], op=OP.subtract)
        nc.scalar.activation(dd[:], dd[:], AF.Exp)
        z2 = sbB.tile([128, 4, 4], F32, tag="z2")
        nc.vector.reduce_sum(z2[:], dd[:].transpose([0, 1, 3, 2]),
                             axis=AX.X)
        rz2 = sbB.tile([128, 4, 4], F32, tag="rz2")
        nc.vector.reciprocal(rz2[:], z2[:])
        nc.vector.tensor_scalar_mul(rz2[:], rz2[:], 1.0 / H)
        nc.vector.tensor_tensor(
            out=dd[:], in0=dd[:],
            in1=rz2[:].unsqueeze(2).to_broadcast((128, 4, A, 4)),
            op=OP.mult)
        ot = sbB.tile([128, 4, A], F32, tag="ot")
        nc.vector.reduce_sum(ot[:], dd[:], axis=AX.X)

        outv = out_ext[t0 * BT:(t0 + 4) * BT, :].rearrange(
            "(t n) a -> n t a", t=4)
        nc.sync.dma_start(out=outv, in_=ot[:])

    for _rep in range(repeat):
        phaseA1(0)
        phaseA1(1)
        phaseA2(0)
        pend = []
        for it in range(NT):
            if it + 2 < NT:
                phaseA1(it + 2)
            if it + 1 < NT:
                phaseA2(it + 1)
            phaseB1(it)
            pend.append(it)
            if len(pend) == 4:
                for t in pend:
                    phaseB2(t)
                pend = []


def build_nc(repeat=1):
    nc = bacc.Bacc(target_bir_lowering=False, trn_type="TRN2")
    x8_ext = nc.declare_dram_parameter("x8", [128, NT, CC, 64], FP8,
                                       isOutput=False)
    xt_ext = nc.declare_dram_parameter("xt", [128, NB // 2, 128], BF16,
                                       isOutput=False)
    out_ext = nc.declare_dram_parameter("out", [NB, A], F32, isOutput=True)
    wext = {}
    for name, (shape, dt) in WSPEC.items():
        wext[name] = nc.declare_dram_parameter(name, list(shape), dt,
                                               isOutput=False)
    with tile.TileContext(nc) as tc:
        with ExitStack() as ctx:
            build_tile_kernel(ctx, tc, x8_ext, xt_ext, out_ext, wext,
                              repeat=repeat)
    nc.compile()
    return nc


def prepare_weights(w_emb, b_emb, w_atlas, b_atlas, w_k, qn_w, qn_b, kn_w,
                    kn_b, lambda_q1, lambda_k1, lambda_q2, lambda_k2):
    import ml_dtypes
    bf = ml_dtypes.bfloat16
    f8 = ml_dtypes.float8_e4m3fn
    f32 = np.float32

    assert np.allclose(b_atlas, 0.0), "kernel assumes b_atlas == 0"
    assert np.allclose(b_emb, 0.0), "kernel assumes b_emb == 0"
    assert np.allclose(kn_b, 0.0), "kernel assumes kn_b == 0"

    # conv1 lhsT [120, 4jp, 2s, 128]: tap 2jp+s; even block cols 3:59,
    # odd block cols 67:123 (the m output partition layout)
    w1 = np.zeros((128, 4, 2, 128), f32)
    w2 = np.zeros((128, 4, 2, 128), f32)
    for t in range(7):
        jp, s = t // 2, t % 2
        blk = np.transpose(w_emb[:, :, t]).astype(f32) * WSCALE
        w1[0:56, jp, s, 3:59] = blk
        w1[64:120, jp, s, 67:123] = blk
        w2[:, jp, s, :] = np.transpose(w_atlas[:, :, t]).astype(f32) * WSCALE

    # k projection with per-head row mean folded out (=> mean_h(k) == 0)
    wkf = np.asarray(w_k, f32)
    wkp = wkf - wkf.reshape(2 * H, HD, E).mean(axis=1, keepdims=True).repeat(
        HD, axis=1).reshape(E, E)
    wkT = np.ascontiguousarray(np.transpose(wkp)).astype(bf)

    G = np.zeros((128, 8), f32)
    for o in range(128):
        G[o, o // HD] = 1.0
    Z24 = np.zeros((128, 24), f32)
    g64 = np.ascontiguousarray(np.concatenate(
        [G, Z24, G, Z24], axis=1)).astype(bf)

    qG = np.ascontiguousarray(
        np.concatenate([G / HD, G], axis=1)).astype(bf)

    expand = np.ascontiguousarray(G.T).astype(f32)            # [8, 128]
    ident128b = np.eye(128, dtype=bf)
    ident64b = np.eye(64, dtype=bf)

    d_idx = np.arange(E) % HD
    c1 = (SCALING * qn_w[d_idx] * kn_w[d_idx]).astype(f32).reshape(128, 1)
    c2 = (SCALING * qn_b[d_idx] * kn_w[d_idx]).astype(f32).reshape(128, 1)

    lam = float(np.exp(np.sum(lambda_q1 * lambda_k1))
                - np.exp(np.sum(lambda_q2 * lambda_k2)) + LAMBDA_INIT)
    lamrow = np.tile(np.array([1.0, lam] * 4, f32), (128, 1))

    wdict = dict(w1=w1.astype(f8), w2=w2.astype(f8), wkT=wkT, g64=g64,
                 qG=qG, expand=expand, ident128b=ident128b,
                 ident64b=ident64b, c1=c1, c2=c2, lamrow=lamrow)
    WSPEC.clear()
    dtmap = {np.dtype(np.float32): F32, np.dtype(bf): BF16,
             np.dtype(f8): FP8}
    for k, v in wdict.items():
        WSPEC[k] = (v.shape, dtmap[v.dtype])
    return wdict


def pack_x(x):
    """x [N, 56, 128] f32 -> (xq8 fp8 [128, N//128, 136, 64],
    xtp bf16 [128, N//2, 128]). Both partition-major, pair q innermost
    for xq8 so DR tap pairs are stride-64 column neighbours."""
    import ml_dtypes
    f8 = ml_dtypes.float8_e4m3fn
    bf = ml_dtypes.bfloat16
    xf = np.asarray(x, np.float32)
    n = xf.shape[0]
    nt = n // 128
    x8 = xf.astype(f8)
    xq8 = np.zeros((128, nt, CC, 64), f8)
    xe = x8[0::2].reshape(nt, 64, A, E)               # [T, q, a, e]
    xo = x8[1::2].reshape(nt, 64, A, E)
    xq8[0:56, :, 3:3 + E, :] = xe.transpose(2, 0, 3, 1)
    xq8[64:120, :, 3:3 + E, :] = xo.transpose(2, 0, 3, 1)
    xb = xf.astype(bf)
    xtp = np.zeros((128, n // 2, 128), bf)
    xtp[:, :, 0:56] = xb[0::2].transpose(2, 0, 1)     # [e, P, a]
    xtp[:, :, 64:120] = xb[1::2].transpose(2, 0, 1)
    return xq8, xtp


_CACHED = {}


def kernel(**inputs):
    xq8, xtp = pack_x(inputs["x"])
    wdict = prepare_weights(
        **{k: np.asarray(v, np.float32) for k, v in inputs.items()
           if k != "x"})
    if "nc" not in _CACHED:
        _CACHED["nc"] = build_nc()
    nc = _CACHED["nc"]
    nbp = NB // 2
    in_maps = []
    for c in range(NCORES):
        m = {"x8": np.ascontiguousarray(xq8[:, c * NT:(c + 1) * NT]),
             "xt": np.ascontiguousarray(xtp[:, c * nbp:(c + 1) * nbp])}
        m.update(wdict)
        in_maps.append(m)
    res = run_bass_kernel_spmd(nc, in_maps, core_ids=list(range(NCORES)))
    return np.concatenate([np.asarray(r["out"]) for r in res.results], axis=0)


if __name__ == "__main__":
    import reference
    inputs = {k: np.asarray(v) for k, v in reference.setup_inputs().items()}
    got = kernel(**inputs)
    exp = np.asarray(reference.reference(**inputs))
    err = np.abs(got - exp).max() / np.abs(exp).max()
    print("rel err:", err)
